# revision 1
# baseline (speedup 1.0000x reference)
"""Sharded multi-head attention for TRN2 (8 NeuronCores).

Problem: B=4, H=16, S=2048, DK=64 attention with boolean mask [B,1,S,S]
(True entries masked out).  The 64 (batch, head) pairs are independent:
core c handles batch c//2, heads (c%2)*8 .. (c%2)*8+8.

Two-stream software pipeline.  The scalar engine (exp) is the
bottleneck (256 x ~1us activations); everything is organized to keep it
fed:
  - TWO independent streams (pairs 0-1 and 2-3) run in alternating
    slots.  Each stream owns half of PSUM: sc [128,1024] (2 banks) +
    accA/accB [65,512] (1 bank each).  A stream's qc/pair boundary
    (epilogue, new accumulators) hides under the other stream's steady
    state, so exp rarely stalls at boundaries.
  - All inputs are DMA'd up front on two queues (qkt on sync; vp+keep
    on gpsimd), priority-ordered so slot 0 can start within ~2us and
    keep[kt] always lands before its first mask-multiply.
  - Epilogue per (head, qc): sums row + acc body copied PSUM->SBUF on
    DVE (frees the PSUM bank for the next qc fast), then
    recip(row partition-0) -> bf16 cast -> gpsimd partition_broadcast
    -> normalize on DVE -> gpsimd DMA out.  Scalar does exp ONLY.
    (gpsimd's Q7 has 1.3-2us op latency: keep it off any chain that
    gates PSUM reuse or DVE progress — only DMAs + the broadcast.)

Per-slot per-stream: exp over the pair tile [128, 1024] (heads A/B
side by side, scale=1/8 folded, no max-subtraction: scores ~ N(0,1));
QK for the next iteration (two 64-contraction matmuls tile-packed in
PE row groups 0:64 / 64:128, running concurrently); mask multiply on
DVE (bf16 2x, keep_T broadcast over the head dim); one lagged PV pair
(V' = [V | ones] so row 64 accumulates softmax denominators).

All DMAs are partition-major with >=2KB contiguous runs (host
pre-swizzles inputs, ones column baked into V').
"""

import numpy as np
import ml_dtypes
from contextlib import ExitStack

import concourse.bass as bass
import concourse.tile as tile
from concourse import bacc, mybir
from concourse.bass_utils import run_bass_kernel_spmd

B, H, S, DK = 4, 16, 2048, 64
N_CORES = 8
HPC = (B * H) // N_CORES  # heads per core = 8
NPAIR = HPC // 2

P = 128            # k-tile size / partition count
NKT = S // P       # 16 k tiles
QCH = 512          # q chunk per head (pair tile = [128, 1024] = 2 PSUM banks)
NQ = S // QCH      # 4 q chunks

BF16 = mybir.dt.bfloat16
F32 = mybir.dt.float32
BF = ml_dtypes.bfloat16

PV_LAG = 1  # PVs issue one stream-slot late (never head-of-queue stalls)


def build_nc():
    nc = bacc.Bacc(None, target_bir_lowering=False)
    # qkt[pair, 0] = [Q_A^T ; Q_B^T] stacked on partitions, [pair, 1] = K
    qkt_ext = nc.declare_dram_parameter("qkt", [NPAIR, 2, P, S], BF16, isOutput=False)
    # vp[h, p, t, :] = [V[h, t*128+p, :], 1.0]
    vp_ext = nc.declare_dram_parameter("vp", [HPC, P, NKT, DK + 1], BF16, isOutput=False)
    # keep[p, t, q] = not mask[q, t*128+p]
    keep_ext = nc.declare_dram_parameter("keep", [P, NKT, S], BF16, isOutput=False)
    # out_T[h, d, q] (host un-transposes)
    out_ext = nc.declare_dram_parameter("outT", [HPC, DK, S], F32, isOutput=True)

    with tile.TileContext(nc) as tc, ExitStack() as ctx:
        singles = ctx.enter_context(tc.tile_pool(name="singles", bufs=1))
        w_pool = ctx.enter_context(tc.tile_pool(name="wp", bufs=3))
        ep_pool = ctx.enter_context(tc.tile_pool(name="ep", bufs=1))
        ps_pool = ctx.enter_context(tc.tile_pool(name="ps", bufs=1, space="PSUM"))

        # ---- persistent SBUF tiles; all inputs prefetched up front ----
        qT, kT, vpt = {}, {}, {}
        for pr in range(NPAIR):
            qT[pr] = singles.tile([P, S], BF16, name=f"qT{pr}")
            kT[pr] = singles.tile([P, S], BF16, name=f"kT{pr}")
        for h in range(HPC):
            vpt[h] = singles.tile([P, NKT, DK + 1], BF16, name=f"vph{h}")
        keep_sb = singles.tile([P, NKT, S], BF16, name="keep_sb")
        ones1 = singles.tile([1, DK], F32, name="ones1")
        nc.gpsimd.memset(ones1, 1.0)

        # DMA issue itself costs ~0.7us of sequencer time per descriptor,
        # so use FEW, BIG transfers: small first chunks to unblock slot 0,
        # then whole-tensor loads.  keep kt_k is needed at ~slot k; pairs
        # 1/3 from slot 64.
        CH = S // 4
        for pr in (0, 2):
            nc.sync.dma_start(out=qT[pr][:, 0:CH], in_=qkt_ext[pr, 0, :, 0:CH])
            nc.sync.dma_start(out=kT[pr][:, 0:CH], in_=qkt_ext[pr, 1, :, 0:CH])
        for pr in (0, 2):
            nc.sync.dma_start(out=kT[pr][:, CH:S], in_=qkt_ext[pr, 1, :, CH:S])
        for pr in (0, 2):
            nc.sync.dma_start(out=qT[pr][:, CH:S], in_=qkt_ext[pr, 0, :, CH:S])
        for pr in (1, 3):
            nc.sync.dma_start(out=kT[pr], in_=qkt_ext[pr, 1])
            nc.sync.dma_start(out=qT[pr], in_=qkt_ext[pr, 0])
        for h in (0, 1, 4, 5):
            nc.gpsimd.dma_start(out=vpt[h], in_=vp_ext[h])
        nc.gpsimd.dma_start(out=keep_sb[:, 0], in_=keep_ext[:, 0])
        nc.gpsimd.dma_start(out=keep_sb[:, 1], in_=keep_ext[:, 1])
        nc.gpsimd.dma_start(out=keep_sb[:, 2:4], in_=keep_ext[:, 2:4])
        nc.gpsimd.dma_start(out=keep_sb[:, 4:8], in_=keep_ext[:, 4:8])
        for h in (2, 3, 6, 7):
            nc.gpsimd.dma_start(out=vpt[h], in_=vp_ext[h])
        nc.gpsimd.dma_start(out=keep_sb[:, 8:NKT], in_=keep_ext[:, 8:NKT])

        # ---- two interleaved streams ----
        streams = []
        for si, prs in enumerate(((0, 1), (2, 3))):
            its = [
                (pr, qc, kt)
                for pr in prs
                for qc in range(NQ)
                for kt in range(NKT)
            ]
            streams.append(
                {"si": si, "iters": its, "sc": None, "accA": None,
                 "accB": None, "pend": [], "dues": []}
            )
        NSLOT = len(streams[0]["iters"])  # 128

        def issue_qk(st, i):
            pr, qc, kt = st["iters"][i]
            si = st["si"]
            q0, k0 = qc * QCH, kt * P
            sc = ps_pool.tile(
                [P, 2 * QCH], F32, tag=f"sc{si}", name=f"sc{si}_{i}", bufs=1
            )
            nc.tensor.matmul(
                sc[:, 0:QCH],
                kT[pr][0:DK, k0 : k0 + P],
                qT[pr][0:DK, q0 : q0 + QCH],
                start=True, stop=True, tile_position=(0, 0),
            )
            nc.tensor.matmul(
                sc[:, QCH : 2 * QCH],
                kT[pr][DK : 2 * DK, k0 : k0 + P],
                qT[pr][DK : 2 * DK, q0 : q0 + QCH],
                start=True, stop=True, tile_position=(64, 0),
            )
            return sc

        def issue_pv(st, ent):
            kt, _, _, w, aA, aB, vA, vB = ent
            nc.tensor.matmul(
                aA, vA[:, kt], w[:, 0:QCH],
                start=(kt == 0), stop=(kt == NKT - 1),
            )
            nc.tensor.matmul(
                aB, vB[:, kt], w[:, QCH : 2 * QCH],
                start=(kt == 0), stop=(kt == NKT - 1),
            )

        def ep_copy(st, acc, tg):
            """free the acc PSUM bank fast: two DVE copies (standard DVE
            ops handle the partition-64 sums row; custom ops do not)"""
            si = st["si"]
            rowF = ep_pool.tile([1, QCH], F32, tag=f"row{si}{tg}", name=f"row{si}{tg}")
            nc.vector.tensor_copy(rowF, acc[DK : DK + 1, :])
            accS = ep_pool.tile([DK, QCH], F32, tag=f"accS{si}{tg}", name=f"accS{si}{tg}")
            nc.vector.tensor_copy(accS, acc[0:DK])
            return rowF, accS

        def ep_norm(st, rowacc, h, qc, tg):
            """recip of sums row, broadcast, normalize, store (off PSUM)"""
            rowF, accS = rowacc
            si = st["si"]
            q0 = qc * QCH
            recipF = ep_pool.tile([1, QCH], F32, tag=f"rF{si}{tg}", name=f"rF{si}{tg}")
            nc.vector.reciprocal_approx_fast(recipF, rowF)
            # fp32 broadcast: skips the bf16 cast (28 x ~0.4us DVE); the
            # doubled gpsimd transfer rides its ample idle time
            bcS = ep_pool.tile([DK, QCH], F32, tag=f"bc{si}{tg}", name=f"bc{si}{tg}")
            nc.gpsimd.partition_broadcast(bcS, recipF)
            outf = ep_pool.tile([DK, QCH], F32, tag=f"of{si}{tg}", name=f"of{si}{tg}")
            nc.vector.tensor_mul(outf, accS, bcS)
            nc.gpsimd.dma_start(out=out_ext[h, :, q0 : q0 + QCH], in_=outf)

        # prologue QKs
        for st in streams:
            st["sc"] = issue_qk(st, 0)

        def run_slot(st, s):
            si = st["si"]
            pr, qc, kt = st["iters"][s]
            if kt == 0:
                st["accA"] = ps_pool.tile(
                    [DK + 1, QCH], F32, tag=f"acc{si}A",
                    name=f"acc{si}A_{pr}_{qc}", bufs=1,
                )
                st["accB"] = ps_pool.tile(
                    [DK + 1, QCH], F32, tag=f"acc{si}B",
                    name=f"acc{si}B_{pr}_{qc}", bufs=1,
                )
            if s % 2 == 0:
                # one w tile spans TWO slots (kt, kt+1); 16 slots per qc
                # is even, so a pair never straddles a qc boundary
                st["wbig"] = w_pool.tile(
                    [P, 4 * QCH], BF16, tag=f"w{si}", name=f"w{si}_{s}"
                )
            w = st["wbig"][:, (s % 2) * 2 * QCH : (s % 2 + 1) * 2 * QCH]
            nc.scalar.activation(
                w, st["sc"], mybir.ActivationFunctionType.Exp, scale=0.125
            )
            if s + 1 < NSLOT:
                st["sc"] = issue_qk(st, s + 1)
            hA, hB = 2 * pr, 2 * pr + 1
            q0 = qc * QCH
            # strictly ONE pop per slot (even 1-PV-pair-per-slot PE
            # cadence), issued BEFORE the mask-mul so a boundary's
            # epilogue copies reach the DVE queue ahead of the 1.4us
            # joint multiply (frees acc PSUM for the new qc's first PV)
            npop = 2 if s >= NSLOT - 4 else 1
            for _ in range(npop):
                if len(st["pend"]) <= PV_LAG:
                    break
                ent = st["pend"].pop(0)
                issue_pv(st, ent)
                if ent[0] == NKT - 1:
                    # epilogue for the finished (pr, qc): free PSUM now,
                    # normalize/store over the next slots
                    _, epr, eqc, _, eA, eB, _, _ = ent

                    def mk(stc, a, b, h0, q):
                        state = {}

                        def c0():
                            state["sA"] = ep_copy(stc, a, "A")
                            state["sB"] = ep_copy(stc, b, "B")

                        def c1():
                            ep_norm(stc, state["sA"], h0, q, "A")

                        def c2():
                            ep_norm(stc, state["sB"], h0 + 1, q, "B")

                        return c0, c1, c2

                    c0, c1, c2 = mk(st, eA, eB, 2 * epr, eqc)
                    c0()
                    st["dues"] += [(s + 1, c1), (s + 2, c2)]
            while st["dues"] and st["dues"][0][0] <= s:
                st["dues"].pop(0)[1]()
            if s % 2 == 1:
                # one masked multiply over both slots and both heads:
                # keep hops kt (stride S) and broadcasts heads (stride 0)
                keep_slice = keep_sb[:, kt - 1, q0 : q0 + QCH]
                keep4 = bass.AP(
                    tensor=keep_slice.tensor,
                    offset=keep_slice.offset,
                    ap=[keep_slice.ap[0], [S, 2], [0, 2], keep_slice.ap[1]],
                )
                w4 = st["wbig"].rearrange("p (t r q) -> p t r q", t=2, r=2)
                nc.vector.tensor_mul(w4, w4, keep4)
                st["pend"].append(
                    (kt - 1, pr, qc, st["wbig"][:, 0 : 2 * QCH],
                     st["accA"], st["accB"], vpt[hA], vpt[hB])
                )
                st["pend"].append(
                    (kt, pr, qc, st["wbig"][:, 2 * QCH : 4 * QCH],
                     st["accA"], st["accB"], vpt[hA], vpt[hB])
                )


        def flush_stream(st):
            """drain this stream's PVs + final epilogue, phase-pipelined
            across its two heads (PE rank-1 broadcast into its dead sc
            banks instead of cast+gpsimd broadcast)"""
            si = st["si"]
            finals = []
            while st["pend"]:
                ent = st["pend"].pop(0)
                issue_pv(st, ent)
                if ent[0] == NKT - 1:
                    _, epr, eqc, _, eA, eB, _, _ = ent
                    finals.append((eA, 2 * epr, eqc, "A"))
                    finals.append((eB, 2 * epr + 1, eqc, "B"))
            for _, fn in st["dues"]:
                fn()
            st["dues"] = []
            # flush needs no fast PSUM free: copy only the sums rows,
            # broadcast via PE into PSUM, hop the broadcast to SBUF with ONE
            # DVE copy (a DVE op may have at most one PSUM operand), then
            # normalize reading acc straight from PSUM
            recips = []
            for acc, h, qc, tg in finals:
                rowF = ep_pool.tile(
                    [1, QCH], F32, tag=f"row{si}{tg}", name=f"rowz{si}{tg}"
                )
                nc.vector.tensor_copy(rowF, acc[DK : DK + 1, :])
                recipF = ep_pool.tile(
                    [1, QCH], F32, tag=f"rF{si}{tg}", name=f"rFz{si}{tg}"
                )
                nc.vector.reciprocal_approx_fast(recipF, rowF)
                recips.append(recipF)
            bcp = ps_pool.tile(
                [DK, 2 * QCH], F32, tag=f"sc{si}", name=f"bcp{si}", bufs=1
            )
            for jj, recipF in enumerate(recips):
                nc.tensor.matmul(
                    bcp[:, jj * QCH : (jj + 1) * QCH],
                    ones1[0:1, 0:DK], recipF, start=True, stop=True,
                )
            bcpS = ep_pool.tile(
                [DK, 2 * QCH], F32, tag=f"bcpS{si}", name=f"bcpS{si}"
            )
            nc.vector.tensor_copy(bcpS, bcp)
            for jj, (acc, h, qc, tg) in enumerate(finals):
                q0 = qc * QCH
                outf = ep_pool.tile(
                    [DK, QCH], F32, tag=f"of{si}{tg}", name=f"ofz{si}{tg}"
                )
                nc.vector.tensor_mul(
                    outf, acc[0:DK], bcpS[:, jj * QCH : (jj + 1) * QCH]
                )
                nc.gpsimd.dma_start(out=out_ext[h, :, q0 : q0 + QCH], in_=outf)

        # STAGGER: stream 1 runs KST slots ahead.  Its head start fits in
        # the DMA-fill startup window (scalar idles there regardless) and
        # its flush epilogue hides under stream 0's last KST slots, so only
        # stream 0's 2-head flush tail is exposed at the very end.
        KST = 6
        for y in range(KST):
            run_slot(streams[1], y)
        for s in range(NSLOT):
            run_slot(streams[0], s)
            sy = s + KST
            if sy < NSLOT:
                run_slot(streams[1], sy)
            elif sy == NSLOT:
                flush_stream(streams[1])
        flush_stream(streams[0])
    nc.finalize()
    return nc


_NC_CACHE = {}


def get_nc():
    if "nc" not in _NC_CACHE:
        _NC_CACHE["nc"] = build_nc()
    return _NC_CACHE["nc"]


def kernel(Q, K, V, mask, _trace=False, _tmpdir=None):
    Q = np.asarray(Q, dtype=np.float32)
    K = np.asarray(K, dtype=np.float32)
    V = np.asarray(V, dtype=np.float32)
    mask = np.asarray(mask)

    in_maps = []
    for c in range(N_CORES):
        b, h0 = c // 2, (c % 2) * HPC
        # [pair, {q,k}, 128, S]: partitions 0:64 = head A dims, 64:128 = head B
        qkt = np.empty((NPAIR, 2, P, S), BF)
        qt = Q[b, h0 : h0 + HPC].transpose(0, 2, 1).reshape(NPAIR, 2 * DK, S)
        kt = K[b, h0 : h0 + HPC].transpose(0, 2, 1).reshape(NPAIR, 2 * DK, S)
        qkt[:, 0] = qt
        qkt[:, 1] = kt
        vp = np.empty((HPC, P, NKT, DK + 1), BF)
        vp[:, :, :, 0:DK] = (
            V[b, h0 : h0 + HPC].reshape(HPC, NKT, P, DK).transpose(0, 2, 1, 3)
        )
        vp[:, :, :, DK] = 1.0
        if c % 2 == 0:
            kp = (~mask[b, 0]).T  # [k, q]
            keep = np.ascontiguousarray(
                kp.reshape(NKT, P, S).transpose(1, 0, 2)
            ).astype(BF)
        in_maps.append({"qkt": qkt, "vp": vp, "keep": keep})

    nc = get_nc()
    res = run_bass_kernel_spmd(
        nc, in_maps, core_ids=list(range(N_CORES)), trace=_trace, tmpdir=_tmpdir
    )
    out = np.empty((B, H, S, DK), np.float32)
    for c in range(N_CORES):
        b, h0 = c // 2, (c % 2) * HPC
        out[b, h0 : h0 + HPC] = np.asarray(res.results[c]["outT"]).transpose(0, 2, 1)
    if _trace:
        return out, res
    return out



# revision 7
# speedup vs baseline: 1.1204x; 1.1204x over previous
"""Sharded multi-head attention for TRN2 (8 NeuronCores).

Problem: B=4, H=16, S=2048, DK=64 attention with boolean mask [B,1,S,S]
(True entries masked out).  The 64 (batch, head) pairs are independent:
core c handles batch c//2, heads (c%2)*8 .. (c%2)*8+8.

Design (v2): single stream, scalar-engine exp is split with the vector
engine, softmax normalization happens on the HOST.

  - Per slot (pr, qc, kt): scores tile sc [128, 1024] f32 in PSUM
    (two heads side by side, k on partitions).  exp is split by q
    column: the first QA columns of each head go through the scalar
    engine (exact exp, ~0.83ns/elem), the rest through the vector
    engine using a Schraudolph bit-trick: u16 = sat(round(s*16/ln2 +
    maskadd)) where maskadd is +16256 (=127<<7, bf16-exact) for kept
    entries and -65536 for masked ones.  Saturation-to-0 of negatives
    makes masked weights exactly +0.0 in bf16, so the additive mask is
    FUSED into the one DVE pass (scalar_tensor_tensor, PSUM f32 in,
    u16 out viewed as bf16).  The ACT part gets a separate
    multiplicative bf16 keep mask on DVE (one tensor_tensor per 2
    slots, 2x mode).  The Schraudolph share only sees ~15-25% of
    elements; its sawtooth error is common-mode-cancelled by softmax
    (verified: rel err 7.6e-3 at QA=436 vs 3.4e-3 exact).
  - PSUM: sc ping-pong (2+2 banks) + acc ping-pong (2+2 banks) = 8.
    QK(s+1) writes the other sc pair while exp(s) drains; PV
    accumulates [V | 1]^T @ w into acc [65, 512] (row 64 = softmax
    denominator); when a (pr, qc) finishes, acc is DMA'd straight
    from PSUM to HBM and the host does num/den.  No on-device
    epilogue at all.
  - DMAs: inputs split across the sync + vector HWDGE rings in exact
    consumption order (masks are qc-outer kt-inner); vp and all
    outputs ride the gpsimd SWDGE queue.
"""

import numpy as np
import ml_dtypes
from contextlib import ExitStack

import concourse.bass as bass
import concourse.tile as tile
from concourse import bacc, mybir
from concourse.bass_utils import run_bass_kernel_spmd

B, H, S, DK = 4, 16, 2048, 64
N_CORES = 8
HPC = (B * H) // N_CORES  # heads per core = 8
NPAIR = HPC // 2

P = 128            # k-tile size / partition count
NKT = S // P       # 16 k tiles
QCH = 512          # q chunk per head (pair tile = [128, 1024] = 2 PSUM banks)
NQ = S // QCH      # 4 q chunks

QA = 436           # q columns per head through scalar-engine exp
QD = QCH - QA      # q columns per head through DVE bit-trick exp

BF16 = mybir.dt.bfloat16
F32 = mybir.dt.float32
U16 = mybir.dt.uint16
BF = ml_dtypes.bfloat16

LN2 = float(np.log(2.0))
SCHRAU_SCALE = 128.0 / LN2 * 0.125   # folds the 1/sqrt(dk)=1/8 score scale
MADD_KEEP = 127.0 * 128.0            # 16256, exactly representable in bf16
MADD_MASK = -65536.0                 # saturates u16 convert to 0 -> +0.0 bf16

PV_LAG = 1


def build_nc():
    nc = bacc.Bacc(None, target_bir_lowering=False)
    # qkt[pair, 0] = [Q_A^T ; Q_B^T] stacked on partitions, [pair, 1] = K
    qkt_ext = nc.declare_dram_parameter("qkt", [NPAIR, 2, P, S], BF16, isOutput=False)
    # vp[h, p, t, :] = [V[h, t*128+p, :], 1.0]
    vp_ext = nc.declare_dram_parameter("vp", [HPC, P, NKT, DK + 1], BF16, isOutput=False)
    # keep_act[p, qc, t, 0:QA] = not mask[qc*512+j, t*128+p]   (ACT part)
    keep_ext = nc.declare_dram_parameter("keepa", [P, NQ, NKT, QA], BF16, isOutput=False)
    # maskadd[p, qc, t, 0:QD]: +16256 kept / -65536 masked       (DVE part)
    madd_ext = nc.declare_dram_parameter("madd", [P, NQ, NKT, QD], BF16, isOutput=False)
    # raw acc dump: [h, qc, 65, 512]; host computes rows0:64 / row64
    out_ext = nc.declare_dram_parameter("outT", [HPC, NQ, DK + 1, QCH], F32, isOutput=True)

    with tile.TileContext(nc) as tc, ExitStack() as ctx:
        singles = ctx.enter_context(tc.tile_pool(name="singles", bufs=1))
        w_pool = ctx.enter_context(tc.tile_pool(name="wp", bufs=3))
        ep_pool = ctx.enter_context(tc.tile_pool(name="ep", bufs=2))
        ps_pool = ctx.enter_context(tc.tile_pool(name="ps", bufs=1, space="PSUM"))

        # ---- persistent SBUF tiles ----
        qT, kT, vpt = {}, {}, {}
        for pr in range(NPAIR):
            qT[pr] = singles.tile([P, S], BF16, name=f"qT{pr}")
            kT[pr] = singles.tile([P, S], BF16, name=f"kT{pr}")
        for h in range(HPC):
            vpt[h] = singles.tile([P, NKT, DK + 1], BF16, name=f"vph{h}")
        keep_sb = singles.tile([P, NQ, NKT, QA], BF16, name="keep_sb")
        madd_sb = singles.tile([P, NQ, NKT, QD], BF16, name="madd_sb")
        dummy = singles.tile([1, 2], F32, name="dummy")

        # warm the ACT exp table during the DMA fill window
        nc.gpsimd.memset(dummy, 0.0)
        nc.scalar.activation(
            dummy, dummy, mybir.ActivationFunctionType.Exp, scale=1.0
        )

        # ---- input DMAs, in consumption order, spread over queues ----
        # iteration is (qc-block, pr, kt): the whole first block (64 slots)
        # uses only qc0 masks, qc0 columns of qT, all of kT, all vp.
        # act-ring DMAs are prologue-only (its sequencer idles pre-exp).
        nc.scalar.dma_start(out=qT[0][:, 0:QCH], in_=qkt_ext[0, 0, :, 0:QCH])
        nc.scalar.dma_start(out=keep_sb[:, 0, 0:4], in_=keep_ext[:, 0, 0:4])
        nc.scalar.dma_start(out=keep_sb[:, 0, 4:8], in_=keep_ext[:, 0, 4:8])
        nc.scalar.dma_start(out=keep_sb[:, 0, 8:12], in_=keep_ext[:, 0, 8:12])
        # sync ring: kT0 + qc0 madd first (slot-0 critical), then the other
        # pairs' kT / qT qc0 chunks by their block start
        nc.sync.dma_start(out=kT[0], in_=qkt_ext[0, 1])
        nc.sync.dma_start(out=madd_sb[:, 0], in_=madd_ext[:, 0])
        nc.sync.dma_start(out=keep_sb[:, 0, 12:NKT], in_=keep_ext[:, 0, 12:NKT])
        for pr in (1, 2, 3):
            nc.sync.dma_start(out=kT[pr], in_=qkt_ext[pr, 1])
            nc.sync.dma_start(out=qT[pr][:, 0:QCH], in_=qkt_ext[pr, 0, :, 0:QCH])
        # later q-chunks + masks per qc block, in consumption order
        for qc in range(1, NQ):
            q0 = qc * QCH
            for pr in range(NPAIR):
                nc.sync.dma_start(
                    out=qT[pr][:, q0 : q0 + QCH],
                    in_=qkt_ext[pr, 0, :, q0 : q0 + QCH],
                )
            nc.sync.dma_start(out=madd_sb[:, qc], in_=madd_ext[:, qc])
            nc.sync.dma_start(out=keep_sb[:, qc, 0:8], in_=keep_ext[:, qc, 0:8])
            nc.sync.dma_start(out=keep_sb[:, qc, 8:NKT], in_=keep_ext[:, qc, 8:NKT])
        # vp on the gpsimd SWDGE queue, per head pair (pair pr from slot 16*pr)
        nc.gpsimd.dma_start(out=vpt[0][:, 0:4], in_=vp_ext[0, :, 0:4])
        nc.gpsimd.dma_start(out=vpt[1][:, 0:4], in_=vp_ext[1, :, 0:4])
        nc.gpsimd.dma_start(out=vpt[0][:, 4:NKT], in_=vp_ext[0, :, 4:NKT])
        nc.gpsimd.dma_start(out=vpt[1][:, 4:NKT], in_=vp_ext[1, :, 4:NKT])
        for h in (2, 3, 4, 5, 6, 7):
            nc.gpsimd.dma_start(out=vpt[h], in_=vp_ext[h])

        iters = [
            (pr, qc, kt)
            for qc in range(NQ)
            for pr in range(NPAIR)
            for kt in range(NKT)
        ]
        NSLOT = len(iters)  # 256

        st = {"sc": None, "accA": None, "accB": None, "pend": [], "wbig": None}

        def issue_qk(i):
            pr, qc, kt = iters[i]
            q0, k0 = qc * QCH, kt * P
            sc = ps_pool.tile(
                [P, 2 * QCH], F32, tag=f"sc{i % 2}", name=f"sc_{i}", bufs=1
            )
            nc.tensor.matmul(
                sc[:, 0:QCH],
                kT[pr][0:DK, k0 : k0 + P],
                qT[pr][0:DK, q0 : q0 + QCH],
                start=True, stop=True, tile_position=(0, 0),
            )
            nc.tensor.matmul(
                sc[:, QCH : 2 * QCH],
                kT[pr][DK : 2 * DK, k0 : k0 + P],
                qT[pr][DK : 2 * DK, q0 : q0 + QCH],
                start=True, stop=True, tile_position=(64, 0),
            )
            return sc

        def issue_pv(ent):
            kt, pr, qc, w, aA, aB, vA, vB = ent
            nc.tensor.matmul(
                aA, vA[:, kt], w[:, 0:QCH],
                start=(kt == 0), stop=(kt == NKT - 1),
            )
            nc.tensor.matmul(
                aB, vB[:, kt], w[:, QCH : 2 * QCH],
                start=(kt == 0), stop=(kt == NKT - 1),
            )
            if kt == NKT - 1:
                # (pr, qc) complete: stage acc in SBUF (frees the PSUM
                # bank), DMA out; host computes rows 0:64 / row 64.
                for acc, h, tg in ((aA, 2 * pr, "A"), (aB, 2 * pr + 1, "B")):
                    accS = ep_pool.tile(
                        [DK + 1, QCH], F32, tag=f"accS{tg}", name=f"accS{tg}_{h}_{qc}"
                    )
                    nc.vector.tensor_copy(accS, acc)
                    nc.gpsimd.dma_start(out=out_ext[h, qc], in_=accS)

        def heads2(ap_tile, off, width, stride=QCH):
            """AP over [128, (2 heads) x width] starting at column off."""
            sl = ap_tile[:, off : off + width]
            return bass.AP(
                tensor=sl.tensor,
                offset=sl.offset,
                ap=[sl.ap[0], [stride, 2], [1, width]],
            )

        def run_slot(s):
            pr, qc, kt = iters[s]
            if kt == 0:
                pp = (qc * NPAIR + pr) % 2
                st["accA"] = ps_pool.tile(
                    [DK + 1, QCH], F32, tag=f"accA{pp}",
                    name=f"accA_{pr}_{qc}", bufs=1,
                )
                st["accB"] = ps_pool.tile(
                    [DK + 1, QCH], F32, tag=f"accB{pp}",
                    name=f"accB_{pr}_{qc}", bufs=1,
                )
            if s % 2 == 0:
                st["wbig"] = w_pool.tile(
                    [P, 4 * QCH], BF16, tag="w", name=f"w_{s}"
                )
            woff = (s % 2) * 2 * QCH
            w = st["wbig"][:, woff : woff + 2 * QCH]
            sc = st["sc"]

            # scalar engine: exact exp on the first QA columns of each head
            nc.scalar.activation(
                heads2(st["wbig"], woff, QA),
                heads2(sc, 0, QA),
                mybir.ActivationFunctionType.Exp,
                scale=0.125,
            )
            if QD > 0:
                # vector engine: Schraudolph bit-trick exp with fused
                # additive mask on the remaining QD columns
                mslice = madd_sb[:, qc, kt, :]
                madd_ap = bass.AP(
                    tensor=mslice.tensor,
                    offset=mslice.offset,
                    ap=[mslice.ap[0], [0, 2], [1, QD]],
                )
                wu = st["wbig"].bitcast(U16)
                nc.vector.scalar_tensor_tensor(
                    heads2(wu, woff + QA, QD),
                    heads2(sc, QA, QD),
                    SCHRAU_SCALE,
                    madd_ap,
                    mybir.AluOpType.mult,
                    mybir.AluOpType.add,
                )
            if s + 1 < NSLOT:
                st["sc"] = issue_qk(s + 1)

            # lagged PV pop: strictly one per slot in steady state
            npop = 2 if s >= NSLOT - 4 else 1
            for _ in range(npop):
                if len(st["pend"]) <= PV_LAG:
                    break
                issue_pv(st["pend"].pop(0))

            if s % 2 == 1:
                # multiplicative keep mask over BOTH slots' ACT columns
                kslice = keep_sb[:, qc, kt - 1, :]
                keep4 = bass.AP(
                    tensor=kslice.tensor,
                    offset=kslice.offset,
                    ap=[kslice.ap[0], [QA, 2], [0, 2], [1, QA]],
                )
                wbase = st["wbig"][:, 0:QA]
                w4 = bass.AP(
                    tensor=wbase.tensor,
                    offset=wbase.offset,
                    ap=[wbase.ap[0], [2 * QCH, 2], [QCH, 2], [1, QA]],
                )
                nc.vector.tensor_mul(w4, w4, keep4)
                hA, hB = 2 * pr, 2 * pr + 1
                st["pend"].append(
                    (kt - 1, pr, qc, st["wbig"][:, 0 : 2 * QCH],
                     st["accA"], st["accB"], vpt[hA], vpt[hB])
                )
                st["pend"].append(
                    (kt, pr, qc, st["wbig"][:, 2 * QCH : 4 * QCH],
                     st["accA"], st["accB"], vpt[hA], vpt[hB])
                )
        st["sc"] = issue_qk(0)
        for s in range(NSLOT):
            run_slot(s)
        while st["pend"]:
            issue_pv(st["pend"].pop(0))
    nc.finalize()
    return nc


_NC_CACHE = {}


def get_nc():
    if "nc" not in _NC_CACHE:
        _NC_CACHE["nc"] = build_nc()
    return _NC_CACHE["nc"]


def kernel(Q, K, V, mask, _trace=False, _tmpdir=None):
    Q = np.asarray(Q, dtype=np.float32)
    K = np.asarray(K, dtype=np.float32)
    V = np.asarray(V, dtype=np.float32)
    mask = np.asarray(mask)

    in_maps = []
    for c in range(N_CORES):
        b, h0 = c // 2, (c % 2) * HPC
        # [pair, {q,k}, 128, S]: partitions 0:64 = head A dims, 64:128 = head B
        qkt = np.empty((NPAIR, 2, P, S), BF)
        qt = Q[b, h0 : h0 + HPC].transpose(0, 2, 1).reshape(NPAIR, 2 * DK, S)
        kt = K[b, h0 : h0 + HPC].transpose(0, 2, 1).reshape(NPAIR, 2 * DK, S)
        qkt[:, 0] = qt
        qkt[:, 1] = kt
        vp = np.empty((HPC, P, NKT, DK + 1), BF)
        vp[:, :, :, 0:DK] = (
            V[b, h0 : h0 + HPC].reshape(HPC, NKT, P, DK).transpose(0, 2, 1, 3)
        )
        vp[:, :, :, DK] = 1.0
        if c % 2 == 0:
            # kp[k, q] = not mask[q, k]; layout [p, qc, kt, q-within-chunk]
            kp = (~mask[b, 0]).T.reshape(NKT, P, NQ, QCH).transpose(1, 2, 0, 3)
            keepa = np.ascontiguousarray(kp[:, :, :, 0:QA]).astype(BF)
            madd = np.where(
                kp[:, :, :, QA:QCH], MADD_KEEP, MADD_MASK
            ).astype(BF)
            madd = np.ascontiguousarray(madd)
        in_maps.append({"qkt": qkt, "vp": vp, "keepa": keepa, "madd": madd})

    nc = get_nc()
    res = run_bass_kernel_spmd(
        nc, in_maps, core_ids=list(range(N_CORES)), trace=_trace, tmpdir=_tmpdir
    )
    out = np.empty((B, H, S, DK), np.float32)
    for c in range(N_CORES):
        b, h0 = c // 2, (c % 2) * HPC
        acc = np.asarray(res.results[c]["outT"])  # [HPC, NQ, DK+1, QCH]
        num = acc[:, :, 0:DK, :]                  # [h, qc, d, q]
        den = acc[:, :, DK, :]                    # [h, qc, q]
        o = num / den[:, :, None, :]
        # -> [h, q, d]
        out[b, h0 : h0 + HPC] = (
            o.transpose(0, 1, 3, 2).reshape(HPC, S, DK)
        )
    if _trace:
        return out, res
    return out


# revision 13
# speedup vs baseline: 1.1547x; 1.0306x over previous
"""Sharded multi-head attention for TRN2 (8 NeuronCores).

Problem: B=4, H=16, S=2048, DK=64 attention with boolean mask [B,1,S,S]
(True entries masked out).  The 64 (batch, head) pairs are independent:
core c handles batch c//2, heads (c%2)*8 .. (c%2)*8+8.

Design (v2): single stream, scalar-engine exp is split with the vector
engine, softmax normalization happens on the HOST.

  - Per slot (pr, qc, kt): scores tile sc [128, 1024] f32 in PSUM
    (two heads side by side, k on partitions).  exp is split by q
    column: the first QA columns of each head go through the scalar
    engine (exact exp, ~0.83ns/elem), the rest through the vector
    engine using a Schraudolph bit-trick: u16 = sat(round(s*16/ln2 +
    maskadd)) where maskadd is +16256 (=127<<7, bf16-exact) for kept
    entries and -65536 for masked ones.  Saturation-to-0 of negatives
    makes masked weights exactly +0.0 in bf16, so the additive mask is
    FUSED into the one DVE pass (scalar_tensor_tensor, PSUM f32 in,
    u16 out viewed as bf16).  The ACT part gets a separate
    multiplicative bf16 keep mask on DVE (one tensor_tensor per 2
    slots, 2x mode).  The Schraudolph share only sees ~15-25% of
    elements; its sawtooth error is common-mode-cancelled by softmax
    (verified: rel err 7.6e-3 at QA=436 vs 3.4e-3 exact).
  - PSUM: sc ping-pong (2+2 banks) + acc ping-pong (2+2 banks) = 8.
    QK(s+1) writes the other sc pair while exp(s) drains; PV
    accumulates [V | 1]^T @ w into acc [65, 512] (row 64 = softmax
    denominator); when a (pr, qc) finishes, acc is DMA'd straight
    from PSUM to HBM and the host does num/den.  No on-device
    epilogue at all.
  - DMAs: inputs split across the sync + vector HWDGE rings in exact
    consumption order (masks are qc-outer kt-inner); vp and all
    outputs ride the gpsimd SWDGE queue.
"""

import numpy as np
import ml_dtypes
from contextlib import ExitStack

import concourse.bass as bass
import concourse.tile as tile
from concourse import bacc, mybir
from concourse.bass_utils import run_bass_kernel_spmd

B, H, S, DK = 4, 16, 2048, 64
N_CORES = 8
HPC = (B * H) // N_CORES  # heads per core = 8
NPAIR = HPC // 2

P = 128            # k-tile size / partition count
NKT = S // P       # 16 k tiles
QCH = 512          # q chunk per head (pair tile = [128, 1024] = 2 PSUM banks)
NQ = S // QCH      # 4 q chunks

QA = 352           # q columns per head through scalar-engine exp
QD = QCH - QA      # q columns per head through DVE bit-trick exp

BF16 = mybir.dt.bfloat16
F32 = mybir.dt.float32
U16 = mybir.dt.uint16
BF = ml_dtypes.bfloat16

LN2 = float(np.log(2.0))
SCHRAU_SCALE = 128.0 / LN2 * 0.125   # folds the 1/sqrt(dk)=1/8 score scale
MADD_KEEP = 127.0 * 128.0            # 16256, exactly representable in bf16
MADD_MASK = -65536.0                 # saturates u16 convert to 0 -> +0.0 bf16

PV_LAG = 2  # pops trail far enough that PV never waits on the DVE mask op


def build_nc():
    nc = bacc.Bacc(None, target_bir_lowering=False)
    # qkt[pair, 0] = [Q_A^T ; Q_B^T] stacked on partitions, [pair, 1] = K
    qkt_ext = nc.declare_dram_parameter("qkt", [NPAIR, 2, P, S], BF16, isOutput=False)
    # vp[h, p, t, :] = [V[h, t*128+p, :], 1.0]
    vp_ext = nc.declare_dram_parameter("vp", [HPC, P, NKT, DK + 1], BF16, isOutput=False)
    # keep_act[p, qc, t, 0:QA] = not mask[qc*512+j, t*128+p]   (ACT part)
    keep_ext = nc.declare_dram_parameter("keepa", [P, NQ, NKT, QA], BF16, isOutput=False)
    # maskadd[p, qc, t, 0:QD]: +16256 kept / -65536 masked       (DVE part)
    madd_ext = nc.declare_dram_parameter("madd", [P, NQ, NKT, QD], BF16, isOutput=False)
    # raw acc dump: [h, qc, 65, 512]; host computes rows0:64 / row64
    out_ext = nc.declare_dram_parameter("outT", [HPC, NQ, DK + 1, QCH], F32, isOutput=True)

    with tile.TileContext(nc) as tc, ExitStack() as ctx:
        singles = ctx.enter_context(tc.tile_pool(name="singles", bufs=1))
        w_pool = ctx.enter_context(tc.tile_pool(name="wp", bufs=4))
        ep_pool = ctx.enter_context(tc.tile_pool(name="ep", bufs=2))
        ps_pool = ctx.enter_context(tc.tile_pool(name="ps", bufs=1, space="PSUM"))

        # ---- persistent SBUF tiles ----
        qT, kT, vpt = {}, {}, {}
        for pr in range(NPAIR):
            qT[pr] = singles.tile([P, S], BF16, name=f"qT{pr}")
            kT[pr] = singles.tile([P, S], BF16, name=f"kT{pr}")
        for h in range(HPC):
            vpt[h] = singles.tile([P, NKT, DK + 1], BF16, name=f"vph{h}")
        keep_sb = singles.tile([P, NQ, NKT, QA], BF16, name="keep_sb")
        madd_sb = singles.tile([P, NQ, NKT, QD], BF16, name="madd_sb")
        dummy = singles.tile([1, 2], F32, name="dummy")

        # warm the ACT exp table during the DMA fill window
        nc.gpsimd.memset(dummy, 0.0)
        nc.scalar.activation(
            dummy, dummy, mybir.ActivationFunctionType.Exp, scale=1.0
        )

        # ---- input DMAs, in consumption order, spread over queues ----
        # iteration is (qc-block, pr, kt): the whole first block (64 slots)
        # uses only qc0 masks, qc0 columns of qT, all of kT, all vp.
        # act-ring DMAs are prologue-only (its sequencer idles pre-exp).
        nc.scalar.dma_start(out=qT[0][:, 0:QCH], in_=qkt_ext[0, 0, :, 0:QCH])
        nc.scalar.dma_start(out=keep_sb[:, 0, 0:4], in_=keep_ext[:, 0, 0:4])
        nc.scalar.dma_start(out=keep_sb[:, 0, 4:8], in_=keep_ext[:, 0, 4:8])
        nc.scalar.dma_start(out=keep_sb[:, 0, 8:12], in_=keep_ext[:, 0, 8:12])
        # sync ring: kT0 kt0-slice + qc0 madd first (slot-0 critical), then
        # the other pairs' kT / qT qc0 chunks by their block start
        nc.sync.dma_start(out=kT[0][:, 0:P], in_=qkt_ext[0, 1, :, 0:P])
        nc.sync.dma_start(out=madd_sb[:, 0, 0:4], in_=madd_ext[:, 0, 0:4])
        nc.sync.dma_start(out=kT[0][:, P:S], in_=qkt_ext[0, 1, :, P:S])
        nc.sync.dma_start(out=madd_sb[:, 0, 4:NKT], in_=madd_ext[:, 0, 4:NKT])
        nc.sync.dma_start(out=keep_sb[:, 0, 12:NKT], in_=keep_ext[:, 0, 12:NKT])
        for pr in (1, 2, 3):
            nc.sync.dma_start(out=kT[pr], in_=qkt_ext[pr, 1])
            nc.sync.dma_start(out=qT[pr][:, 0:QCH], in_=qkt_ext[pr, 0, :, 0:QCH])
        # later q-chunks + masks per qc block, in consumption order
        for qc in range(1, NQ):
            q0 = qc * QCH
            for pr in range(NPAIR):
                nc.sync.dma_start(
                    out=qT[pr][:, q0 : q0 + QCH],
                    in_=qkt_ext[pr, 0, :, q0 : q0 + QCH],
                )
            nc.sync.dma_start(out=madd_sb[:, qc], in_=madd_ext[:, qc])
            nc.sync.dma_start(out=keep_sb[:, qc, 0:8], in_=keep_ext[:, qc, 0:8])
            nc.sync.dma_start(out=keep_sb[:, qc, 8:NKT], in_=keep_ext[:, qc, 8:NKT])
        # vp on the gpsimd SWDGE queue, per head pair (pair pr from slot 16*pr)
        nc.gpsimd.dma_start(out=vpt[0][:, 0:4], in_=vp_ext[0, :, 0:4])
        nc.gpsimd.dma_start(out=vpt[1][:, 0:4], in_=vp_ext[1, :, 0:4])
        nc.gpsimd.dma_start(out=vpt[0][:, 4:NKT], in_=vp_ext[0, :, 4:NKT])
        nc.gpsimd.dma_start(out=vpt[1][:, 4:NKT], in_=vp_ext[1, :, 4:NKT])
        for h in (2, 3, 4, 5, 6, 7):
            nc.gpsimd.dma_start(out=vpt[h], in_=vp_ext[h])

        iters = [
            (pr, qc, kt)
            for qc in range(NQ)
            for pr in range(NPAIR)
            for kt in range(NKT)
        ]
        NSLOT = len(iters)  # 256

        st = {"sc": None, "accA": None, "accB": None, "pend": [], "wbig": None}

        def issue_qk(i):
            pr, qc, kt = iters[i]
            q0, k0 = qc * QCH, kt * P
            sc = ps_pool.tile(
                [P, 2 * QCH], F32, tag=f"sc{i % 2}", name=f"sc_{i}", bufs=1
            )
            nc.tensor.matmul(
                sc[:, 0:QCH],
                kT[pr][0:DK, k0 : k0 + P],
                qT[pr][0:DK, q0 : q0 + QCH],
                start=True, stop=True, tile_position=(0, 0),
            )
            nc.tensor.matmul(
                sc[:, QCH : 2 * QCH],
                kT[pr][DK : 2 * DK, k0 : k0 + P],
                qT[pr][DK : 2 * DK, q0 : q0 + QCH],
                start=True, stop=True, tile_position=(64, 0),
            )
            return sc

        def issue_pv(ent):
            kt, pr, qc, w, aA, aB, vA, vB = ent
            nc.tensor.matmul(
                aA, vA[:, kt], w[:, 0:QCH],
                start=(kt == 0), stop=(kt == NKT - 1),
            )
            nc.tensor.matmul(
                aB, vB[:, kt], w[:, QCH : 2 * QCH],
                start=(kt == 0), stop=(kt == NKT - 1),
            )
            if kt == NKT - 1:
                # (pr, qc) complete: stage accs in SBUF via scalar-engine
                # copies (frees the PSUM banks), DMA out; host divides.
                for acc, h, tg in ((aA, 2 * pr, "A"), (aB, 2 * pr + 1, "B")):
                    accS = ep_pool.tile(
                        [DK + 1, QCH], F32, tag=f"accS{tg}", name=f"accS{tg}_{h}_{qc}"
                    )
                    nc.scalar.copy(accS, acc)
                    nc.gpsimd.dma_start(out=out_ext[h, qc], in_=accS)

        def heads2(ap_tile, off, width, stride=QCH):
            """AP over [128, (2 heads) x width] starting at column off."""
            sl = ap_tile[:, off : off + width]
            return bass.AP(
                tensor=sl.tensor,
                offset=sl.offset,
                ap=[sl.ap[0], [stride, 2], [1, width]],
            )

        def run_slot(s):
            pr, qc, kt = iters[s]
            if kt == 0:
                pp = (qc * NPAIR + pr) % 2
                st["accA"] = ps_pool.tile(
                    [DK + 1, QCH], F32, tag=f"accA{pp}",
                    name=f"accA_{pr}_{qc}", bufs=1,
                )
                st["accB"] = ps_pool.tile(
                    [DK + 1, QCH], F32, tag=f"accB{pp}",
                    name=f"accB_{pr}_{qc}", bufs=1,
                )
            if s % 2 == 0:
                st["wbig"] = w_pool.tile(
                    [P, 4 * QCH], BF16, tag="w", name=f"w_{s}"
                )
            woff = (s % 2) * 2 * QCH
            w = st["wbig"][:, woff : woff + 2 * QCH]
            sc = st["sc"]

            # scalar engine: exact exp on the first QA columns of each head
            nc.scalar.activation(
                heads2(st["wbig"], woff, QA),
                heads2(sc, 0, QA),
                mybir.ActivationFunctionType.Exp,
                scale=0.125,
            )
            if QD > 0:
                # vector engine: Schraudolph bit-trick exp with fused
                # additive mask on the remaining QD columns
                mslice = madd_sb[:, qc, kt, :]
                madd_ap = bass.AP(
                    tensor=mslice.tensor,
                    offset=mslice.offset,
                    ap=[mslice.ap[0], [0, 2], [1, QD]],
                )
                wu = st["wbig"].bitcast(U16)
                nc.vector.scalar_tensor_tensor(
                    heads2(wu, woff + QA, QD),
                    heads2(sc, QA, QD),
                    SCHRAU_SCALE,
                    madd_ap,
                    mybir.AluOpType.mult,
                    mybir.AluOpType.add,
                )
            if s + 1 < NSLOT:
                st["sc"] = issue_qk(s + 1)

            # lagged PV pop: strictly one per slot in steady state
            npop = 2 if s >= NSLOT - 4 else 1
            for _ in range(npop):
                if len(st["pend"]) <= PV_LAG:
                    break
                issue_pv(st["pend"].pop(0))

            if s % 2 == 1:
                # multiplicative keep mask over BOTH slots' ACT columns
                kslice = keep_sb[:, qc, kt - 1, :]
                keep4 = bass.AP(
                    tensor=kslice.tensor,
                    offset=kslice.offset,
                    ap=[kslice.ap[0], [QA, 2], [0, 2], [1, QA]],
                )
                wbase = st["wbig"][:, 0:QA]
                w4 = bass.AP(
                    tensor=wbase.tensor,
                    offset=wbase.offset,
                    ap=[wbase.ap[0], [2 * QCH, 2], [QCH, 2], [1, QA]],
                )
                nc.vector.tensor_mul(w4, w4, keep4)
                hA, hB = 2 * pr, 2 * pr + 1
                st["pend"].append(
                    (kt - 1, pr, qc, st["wbig"][:, 0 : 2 * QCH],
                     st["accA"], st["accB"], vpt[hA], vpt[hB])
                )
                st["pend"].append(
                    (kt, pr, qc, st["wbig"][:, 2 * QCH : 4 * QCH],
                     st["accA"], st["accB"], vpt[hA], vpt[hB])
                )
        st["sc"] = issue_qk(0)
        for s in range(NSLOT):
            run_slot(s)
        while st["pend"]:
            issue_pv(st["pend"].pop(0))
    nc.finalize()
    return nc


_NC_CACHE = {}


def get_nc():
    if "nc" not in _NC_CACHE:
        _NC_CACHE["nc"] = build_nc()
    return _NC_CACHE["nc"]


def kernel(Q, K, V, mask, _trace=False, _tmpdir=None):
    Q = np.asarray(Q, dtype=np.float32)
    K = np.asarray(K, dtype=np.float32)
    V = np.asarray(V, dtype=np.float32)
    mask = np.asarray(mask)

    in_maps = []
    for c in range(N_CORES):
        b, h0 = c // 2, (c % 2) * HPC
        # [pair, {q,k}, 128, S]: partitions 0:64 = head A dims, 64:128 = head B
        qkt = np.empty((NPAIR, 2, P, S), BF)
        qt = Q[b, h0 : h0 + HPC].transpose(0, 2, 1).reshape(NPAIR, 2 * DK, S)
        kt = K[b, h0 : h0 + HPC].transpose(0, 2, 1).reshape(NPAIR, 2 * DK, S)
        qkt[:, 0] = qt
        qkt[:, 1] = kt
        vp = np.empty((HPC, P, NKT, DK + 1), BF)
        vp[:, :, :, 0:DK] = (
            V[b, h0 : h0 + HPC].reshape(HPC, NKT, P, DK).transpose(0, 2, 1, 3)
        )
        vp[:, :, :, DK] = 1.0
        if c % 2 == 0:
            # kp[k, q] = not mask[q, k]; layout [p, qc, kt, q-within-chunk]
            kp = (~mask[b, 0]).T.reshape(NKT, P, NQ, QCH).transpose(1, 2, 0, 3)
            keepa = np.ascontiguousarray(kp[:, :, :, 0:QA]).astype(BF)
            madd = np.where(
                kp[:, :, :, QA:QCH], MADD_KEEP, MADD_MASK
            ).astype(BF)
            madd = np.ascontiguousarray(madd)
        in_maps.append({"qkt": qkt, "vp": vp, "keepa": keepa, "madd": madd})

    nc = get_nc()
    res = run_bass_kernel_spmd(
        nc, in_maps, core_ids=list(range(N_CORES)), trace=_trace, tmpdir=_tmpdir
    )
    out = np.empty((B, H, S, DK), np.float32)
    for c in range(N_CORES):
        b, h0 = c // 2, (c % 2) * HPC
        acc = np.asarray(res.results[c]["outT"])  # [HPC, NQ, DK+1, QCH]
        num = acc[:, :, 0:DK, :]                  # [h, qc, d, q]
        den = acc[:, :, DK, :]                    # [h, qc, q]
        o = num / den[:, :, None, :]
        # -> [h, q, d]
        out[b, h0 : h0 + HPC] = (
            o.transpose(0, 1, 3, 2).reshape(HPC, S, DK)
        )
    if _trace:
        return out, res
    return out
